# revision 5
# baseline (speedup 1.0000x reference)
"""Trainium2 Bass kernel for nn_InformationAggregationLayer.

Math (per batch b, reference semantics):
  wc_out = cur @ wc_w.T + wc_b                      [S, D]
  wh_out = hist_h @ wh_w.T + wh_b                   [S, D]  per h
  comb   = sigmoid(wc_out + wh_out)                 [S, D]
  alpha  = comb @ qt_w.T + qt_b                     [S, 1]
  HS     = sum_h alpha_h * hist_h                   [S, D]
  out    = cur @ wf1.T + HS @ wf2.T + wf_b          [S, E]
         (wf1 = wf_w[:, :D], wf2 = wf_w[:, D:])

Device strategy (one NeuronCore per batch, 8 cores):
  - All tensors on chip in TRANSPOSED layout [feature, seq] so the
    feature contraction dim sits on SBUF partitions for matmuls.
  - hist fed pre-transposed from host: histT [H, D, S].
  - Per h: PE computes wh_outT into PSUM (fp32r), injects the
    precomputed (wc_out + wc_b + wh_b)T via an identity matmul into the
    same accumulation group, ACT applies sigmoid PSUM->SBUF, PE contracts
    with a broadcast-stationary qt (every output partition gets the same
    alpha row), then DVE/Pool compute HS += alpha * histT.
  - qt_b is folded into a host-precomputed epilogue constant:
      sum_h (a_h + qt_b) * hist_h @ wf2.T
        = [sum_h a_h*hist_h] @ wf2.T + qt_b * (sum_h hist_h) @ wf2.T,
    and the second term (+ wf_b) is c0T, uploaded per batch.
"""

import numpy as np

B, H, S, D, E = 8, 50, 512, 256, 128
P = 128
N_CORES = 8

_PROG_CACHE = {}

# Engine split for the weighted-sum accumulate: columns [0:ACC_SPLIT) of the
# [P, 2*S] acc tile are added on DVE, the rest on GpSimd (Pool).
ACC_SPLIT = 256


def _build_program():
    import concourse.bacc as bacc
    import concourse.tile as tile
    import concourse.mybir as mybir

    FP = mybir.dt.float32
    FPR = mybir.dt.float32r
    SIG = mybir.ActivationFunctionType.Sigmoid
    ADD = mybir.AluOpType.add
    MUL = mybir.AluOpType.mult

    nc = bacc.Bacc("TRN2", target_bir_lowering=False, debug=False,
                   num_devices=N_CORES)

    # Per-core inputs. histT is viewed [H, 2, 128, S] so each partition of the
    # on-chip [128, 2*S] tile pulls its two d-slices. Matmul operands are
    # float32r (1 cycle/row on the PE vs 4 for plain fp32); walrus requires
    # every producer of an fp32r matmul operand to emit fp32r.
    histT = nc.dram_tensor("histT", [H, 2, P, S], FPR, kind="ExternalInput")
    curT = nc.dram_tensor("curT", [D, S], FPR, kind="ExternalInput")
    whT = nc.dram_tensor("whT", [D, D], FPR, kind="ExternalInput")
    wcT = nc.dram_tensor("wcT", [D, D], FPR, kind="ExternalInput")
    qtB = nc.dram_tensor("qtB", [D, P], FPR, kind="ExternalInput")
    wf1T = nc.dram_tensor("wf1T", [D, E], FPR, kind="ExternalInput")
    wf2T = nc.dram_tensor("wf2T", [D, E], FPR, kind="ExternalInput")
    biasCW = nc.dram_tensor("biasCW", [D, 1], FP, kind="ExternalInput")
    ident = nc.dram_tensor("ident", [P, P], FPR, kind="ExternalInput")
    c0T = nc.dram_tensor("c0T", [E, S], FP, kind="ExternalInput")
    outT = nc.dram_tensor("outT", [E, S], FP, kind="ExternalOutput")

    with tile.TileContext(nc) as tc:
        with (
            tc.tile_pool(name="w", bufs=1) as wp,
            tc.tile_pool(name="hh", bufs=3) as hhp,
            tc.tile_pool(name="comb", bufs=2) as combp,
            tc.tile_pool(name="tmp", bufs=3) as tmpp,
            tc.tile_pool(name="acc", bufs=1) as accp,
            tc.tile_pool(name="pwh", bufs=2, space="PSUM") as pwh,
            tc.tile_pool(name="pal", bufs=2, space="PSUM") as palp,
        ):
            # ---- weights / constants to SBUF -------------------------------
            def load(name, dram_ap, shape, dt=FPR):
                t = wp.tile(shape, dt, tag=name)
                nc.sync.dma_start(t[:], dram_ap)
                return t

            whT_t = [load(f"whT{k}", whT[k * P:(k + 1) * P, :], [P, D])
                     for k in range(2)]
            wcT_t = [load(f"wcT{k}", wcT[k * P:(k + 1) * P, :], [P, D])
                     for k in range(2)]
            qtB_t = [load(f"qtB{k}", qtB[k * P:(k + 1) * P, :], [P, P])
                     for k in range(2)]
            wf1_t = [load(f"wf1{k}", wf1T[k * P:(k + 1) * P, :], [P, E])
                     for k in range(2)]
            wf2_t = [load(f"wf2{k}", wf2T[k * P:(k + 1) * P, :], [P, E])
                     for k in range(2)]
            cur_t = [load(f"cur{k}", curT[k * P:(k + 1) * P, :], [P, S])
                     for k in range(2)]
            bias_t = [load(f"bias{k}", biasCW[k * P:(k + 1) * P, :], [P, 1],
                           dt=FP)
                      for k in range(2)]
            id_t = load("ident", ident[:, :], [P, P])
            c0_t = load("c0T", c0T[:, :], [E, S], dt=FP)

            acc = accp.tile([P, 2 * S], FPR, tag="acc")
            nc.gpsimd.memset(acc[:].bitcast(FP), 0.0)

            # ---- prologue: wcoT = (cur @ wc.T + wc_b + wh_b)T --------------
            # transposed layout [dout, s]; m-th dout-tile lives in free cols
            # [m*S, (m+1)*S).
            wcps = pwh.tile([P, 2 * S], FP, tag="ps")
            for m in range(2):
                for k in range(2):
                    nc.tensor.matmul(
                        wcps[:, m * S:(m + 1) * S],
                        wcT_t[k][:, m * P:(m + 1) * P],
                        cur_t[k][:],
                        start=(k == 0), stop=(k == 1),
                    )
            wco = wp.tile([P, 2 * S], FPR, tag="wco")
            for m in range(2):
                sl = slice(m * S, (m + 1) * S)
                nc.vector.tensor_scalar(
                    out=wco[:, sl], in0=wcps[:, sl],
                    scalar1=bias_t[m][:], scalar2=None, op0=ADD,
                )

            # ---- main loop over history ------------------------------------
            for h in range(H):
                hh = hhp.tile([P, 2 * S], FPR, tag="hh")
                # [2, 128, S] dram -> [128, 2, S] on chip
                nc.sync.dma_start(
                    hh[:].rearrange("p (k s) -> p k s", k=2),
                    histT[h].rearrange("k p s -> p k s"),
                )
                ps = pwh.tile([P, 2 * S], FP, tag="ps")
                for m in range(2):
                    sl = slice(m * S, (m + 1) * S)
                    nc.tensor.matmul(
                        ps[:, sl],
                        whT_t[0][:, m * P:(m + 1) * P],
                        hh[:, 0:S],
                        start=True, stop=False,
                    )
                    nc.tensor.matmul(
                        ps[:, sl],
                        whT_t[1][:, m * P:(m + 1) * P],
                        hh[:, S:2 * S],
                        start=False, stop=False,
                    )
                    # inject wc_out + biases (identity stationary)
                    nc.tensor.matmul(
                        ps[:, sl],
                        id_t[:],
                        wco[:, sl],
                        start=False, stop=True,
                    )
                comb = combp.tile([P, 2 * S], FPR, tag="comb")
                nc.scalar.activation(comb[:], ps[:], SIG)

                # alpha (without qt_b), broadcast to all 128 partitions:
                # stationary qtB[k][i, m] = qt[k*128+i] for every m.
                pal = palp.tile([P, S], FP, tag="pal")
                nc.tensor.matmul(pal[:], qtB_t[0][:], comb[:, 0:S],
                                 start=True, stop=False)
                nc.tensor.matmul(pal[:], qtB_t[1][:], comb[:, S:2 * S],
                                 start=False, stop=True)

                # weighted sum: acc += alpha * histT
                tmp = tmpp.tile([P, 2 * S], FPR, tag="tmp")
                nc.vector.tensor_tensor(
                    out=tmp[:].rearrange("p (k s) -> p k s", k=2),
                    in0=hh[:].rearrange("p (k s) -> p k s", k=2),
                    in1=pal[:].unsqueeze(1).broadcast_to((P, 2, S)),
                    op=MUL,
                )
                nc.vector.tensor_tensor(
                    out=acc[:, 0:ACC_SPLIT], in0=acc[:, 0:ACC_SPLIT],
                    in1=tmp[:, 0:ACC_SPLIT], op=ADD)
                nc.gpsimd.tensor_tensor(
                    out=acc[:, ACC_SPLIT:], in0=acc[:, ACC_SPLIT:],
                    in1=tmp[:, ACC_SPLIT:], op=ADD)

            # ---- epilogue ---------------------------------------------------
            pfin = palp.tile([E, S], FP, tag="pal")
            nc.tensor.matmul(pfin[:], wf1_t[0][:], cur_t[0][:],
                             start=True, stop=False)
            nc.tensor.matmul(pfin[:], wf1_t[1][:], cur_t[1][:],
                             start=False, stop=False)
            nc.tensor.matmul(pfin[:], wf2_t[0][:], acc[:, 0:S],
                             start=False, stop=False)
            nc.tensor.matmul(pfin[:], wf2_t[1][:], acc[:, S:2 * S],
                             start=False, stop=True)
            osb = wp.tile([E, S], FP, tag="osb")
            nc.vector.tensor_tensor(out=osb[:], in0=pfin[:], in1=c0_t[:],
                                    op=ADD)
            nc.sync.dma_start(outT[:, :], osb[:])

    nc.compile()
    return nc


def _get_program():
    if "nc" not in _PROG_CACHE:
        _PROG_CACHE["nc"] = _build_program()
    return _PROG_CACHE["nc"]


def prepare_in_maps(wca_history_embeddings, wca_current_embeddings,
                    wc_w, wc_b, wh_w, wh_b, qt_w, qt_b, wf_w, wf_b):
    hist = np.asarray(wca_history_embeddings, dtype=np.float32)
    cur = np.asarray(wca_current_embeddings, dtype=np.float32)
    f32 = lambda x: np.ascontiguousarray(np.asarray(x), dtype=np.float32)

    whT = f32(np.asarray(wh_w).T)
    wcT = f32(np.asarray(wc_w).T)
    qtB = f32(np.repeat(np.asarray(qt_w).reshape(D, 1), P, axis=1))
    wf1T = f32(np.asarray(wf_w)[:, :D].T)
    wf2T = f32(np.asarray(wf_w)[:, D:].T)
    biasCW = f32((np.asarray(wc_b) + np.asarray(wh_b)).reshape(D, 1))
    identity = f32(np.eye(P))
    qtb = float(np.asarray(qt_b).reshape(-1)[0])
    wfb = np.asarray(wf_b, dtype=np.float64)

    in_maps = []
    for b in range(B):
        histT_b = np.ascontiguousarray(
            hist[b].transpose(0, 2, 1), dtype=np.float32
        ).reshape(H, 2, P, S)
        curT_b = f32(cur[b].T)
        hs0 = hist[b].astype(np.float64).sum(axis=0)          # [S, D]
        c0 = qtb * (hs0 @ wf2T.astype(np.float64)) + wfb      # [S, E]
        c0T_b = f32(c0.T)
        in_maps.append({
            "histT": histT_b, "curT": curT_b, "whT": whT, "wcT": wcT,
            "qtB": qtB, "wf1T": wf1T, "wf2T": wf2T, "biasCW": biasCW,
            "ident": identity, "c0T": c0T_b,
        })
    return in_maps


def run_on_device(in_maps, trace=False, **kwargs):
    from concourse import bass_utils
    nc = _get_program()
    res = bass_utils.run_bass_kernel_spmd(
        nc, in_maps, core_ids=list(range(N_CORES)), trace=trace, **kwargs)
    return res


def kernel(wca_history_embeddings, wca_current_embeddings,
           wc_w, wc_b, wh_w, wh_b, qt_w, qt_b, wf_w, wf_b):
    in_maps = prepare_in_maps(
        wca_history_embeddings, wca_current_embeddings,
        wc_w, wc_b, wh_w, wh_b, qt_w, qt_b, wf_w, wf_b)
    res = run_on_device(in_maps)
    out = np.empty((B, S, E), dtype=np.float32)
    for b in range(B):
        out[b] = res.results[b]["outT"].T
    return out


# revision 6
# speedup vs baseline: 1.1286x; 1.1286x over previous
"""Trainium2 Bass kernel for nn_InformationAggregationLayer.

Math (per batch b, reference semantics):
  wc_out = cur @ wc_w.T + wc_b                      [S, D]
  wh_out = hist_h @ wh_w.T + wh_b                   [S, D]  per h
  comb   = sigmoid(wc_out + wh_out)                 [S, D]
  alpha  = comb @ qt_w.T + qt_b                     [S, 1]
  HS     = sum_h alpha_h * hist_h                   [S, D]
  out    = cur @ wf1.T + HS @ wf2.T + wf_b          [S, E]
         (wf1 = wf_w[:, :D], wf2 = wf_w[:, D:])

Device strategy (one NeuronCore per batch, 8 cores):
  - All tensors on chip in TRANSPOSED layout [feature, seq] so the
    feature contraction dim sits on SBUF partitions for matmuls.
  - hist fed pre-transposed from host: histT [H, D, S].
  - Per h: PE computes wh_outT into PSUM (fp32r), injects the
    precomputed (wc_out + wc_b + wh_b)T via an identity matmul into the
    same accumulation group, ACT applies sigmoid PSUM->SBUF, PE contracts
    with a broadcast-stationary qt (every output partition gets the same
    alpha row), then DVE/Pool compute HS += alpha * histT.
  - qt_b is folded into a host-precomputed epilogue constant:
      sum_h (a_h + qt_b) * hist_h @ wf2.T
        = [sum_h a_h*hist_h] @ wf2.T + qt_b * (sum_h hist_h) @ wf2.T,
    and the second term (+ wf_b) is c0T, uploaded per batch.
"""

import numpy as np

B, H, S, D, E = 8, 50, 512, 256, 128
P = 128
N_CORES = 8

_PROG_CACHE = {}

# Engine split for the weighted-sum accumulate: columns [0:ACC_SPLIT) of the
# [P, 2*S] acc tile are added on DVE, the rest on GpSimd (Pool).
ACC_SPLIT = 256


def _build_program():
    import concourse.bacc as bacc
    import concourse.tile as tile
    import concourse.mybir as mybir

    FP = mybir.dt.float32
    FPR = mybir.dt.float32r
    BF = mybir.dt.bfloat16
    SIG = mybir.ActivationFunctionType.Sigmoid
    ADD = mybir.AluOpType.add
    MUL = mybir.AluOpType.mult

    nc = bacc.Bacc("TRN2", target_bir_lowering=False, debug=False,
                   num_devices=N_CORES)

    # Per-core inputs. histT is viewed [H, 2, 128, S] so each partition of the
    # on-chip [128, 2*S] tile pulls its two d-slices. Matmul operands are
    # float32r (1 cycle/row on the PE vs 4 for plain fp32); walrus requires
    # every producer of an fp32r matmul operand to emit fp32r.
    histT = nc.dram_tensor("histT", [H, 2, P, S], BF, kind="ExternalInput")
    curT = nc.dram_tensor("curT", [D, S], FPR, kind="ExternalInput")
    whT = nc.dram_tensor("whT", [D, D], BF, kind="ExternalInput")
    wcT = nc.dram_tensor("wcT", [D, D], FPR, kind="ExternalInput")
    qtB = nc.dram_tensor("qtB", [D, P], BF, kind="ExternalInput")
    wf1T = nc.dram_tensor("wf1T", [D, E], FPR, kind="ExternalInput")
    wf2T = nc.dram_tensor("wf2T", [D, E], FPR, kind="ExternalInput")
    biasCW = nc.dram_tensor("biasCW", [D, 1], FP, kind="ExternalInput")
    ident = nc.dram_tensor("ident", [P, P], BF, kind="ExternalInput")
    c0T = nc.dram_tensor("c0T", [E, S], FP, kind="ExternalInput")
    outT = nc.dram_tensor("outT", [E, S], FP, kind="ExternalOutput")

    with tile.TileContext(nc) as tc:
        with (
            tc.tile_pool(name="w", bufs=1) as wp,
            tc.tile_pool(name="hh", bufs=3) as hhp,
            tc.tile_pool(name="comb", bufs=2) as combp,
            tc.tile_pool(name="tmp", bufs=3) as tmpp,
            tc.tile_pool(name="acc", bufs=1) as accp,
            tc.tile_pool(name="pwh", bufs=3, space="PSUM") as pwh,
            tc.tile_pool(name="pal", bufs=2, space="PSUM") as palp,
        ):
            # ---- weights / constants to SBUF -------------------------------
            def load(name, dram_ap, shape, dt=FPR):
                t = wp.tile(shape, dt, tag=name)
                nc.sync.dma_start(t[:], dram_ap)
                return t

            whT_t = [load(f"whT{k}", whT[k * P:(k + 1) * P, :], [P, D], dt=BF)
                     for k in range(2)]
            wcT_t = [load(f"wcT{k}", wcT[k * P:(k + 1) * P, :], [P, D])
                     for k in range(2)]
            qtB_t = [load(f"qtB{k}", qtB[k * P:(k + 1) * P, :], [P, P], dt=BF)
                     for k in range(2)]
            wf1_t = [load(f"wf1{k}", wf1T[k * P:(k + 1) * P, :], [P, E])
                     for k in range(2)]
            wf2_t = [load(f"wf2{k}", wf2T[k * P:(k + 1) * P, :], [P, E])
                     for k in range(2)]
            cur_t = [load(f"cur{k}", curT[k * P:(k + 1) * P, :], [P, S])
                     for k in range(2)]
            bias_t = [load(f"bias{k}", biasCW[k * P:(k + 1) * P, :], [P, 1],
                           dt=FP)
                      for k in range(2)]
            id_t = load("ident", ident[:, :], [P, P], dt=BF)
            c0_t = load("c0T", c0T[:, :], [E, S], dt=FP)

            acc = accp.tile([P, 2 * S], FPR, tag="acc")
            nc.gpsimd.memset(acc[:].bitcast(FP), 0.0)

            # ---- prologue: wcoT = (cur @ wc.T + wc_b + wh_b)T --------------
            # transposed layout [dout, s]; m-th dout-tile lives in free cols
            # [m*S, (m+1)*S).
            wcps = pwh.tile([P, 2 * S], FP, tag="ps")
            for m in range(2):
                for k in range(2):
                    nc.tensor.matmul(
                        wcps[:, m * S:(m + 1) * S],
                        wcT_t[k][:, m * P:(m + 1) * P],
                        cur_t[k][:],
                        start=(k == 0), stop=(k == 1),
                    )
            wco = wp.tile([P, 2 * S], BF, tag="wco")
            for m in range(2):
                sl = slice(m * S, (m + 1) * S)
                nc.vector.tensor_scalar(
                    out=wco[:, sl], in0=wcps[:, sl],
                    scalar1=bias_t[m][:], scalar2=None, op0=ADD,
                )

            # ---- main loop over history ------------------------------------
            for h in range(H):
                hh = hhp.tile([P, 2 * S], BF, tag="hh")
                nc.sync.dma_start(hh[:, 0:S], histT[h, 0])
                nc.sync.dma_start(hh[:, S:2 * S], histT[h, 1])
                ps = pwh.tile([P, 2 * S], FP, tag="ps")
                for m in range(2):
                    sl = slice(m * S, (m + 1) * S)
                    nc.tensor.matmul(
                        ps[:, sl],
                        whT_t[0][:, m * P:(m + 1) * P],
                        hh[:, 0:S],
                        start=True, stop=False,
                    )
                    nc.tensor.matmul(
                        ps[:, sl],
                        whT_t[1][:, m * P:(m + 1) * P],
                        hh[:, S:2 * S],
                        start=False, stop=False,
                    )
                    # inject wc_out + biases (identity stationary)
                    nc.tensor.matmul(
                        ps[:, sl],
                        id_t[:],
                        wco[:, sl],
                        start=False, stop=True,
                    )
                comb = combp.tile([P, 2 * S], BF, tag="comb")
                nc.scalar.activation(comb[:], ps[:], SIG)

                # alpha (without qt_b), broadcast to all 128 partitions:
                # stationary qtB[k][i, m] = qt[k*128+i] for every m.
                pal = palp.tile([P, S], FP, tag="pal")
                nc.tensor.matmul(pal[:], qtB_t[0][:], comb[:, 0:S],
                                 start=True, stop=False)
                nc.tensor.matmul(pal[:], qtB_t[1][:], comb[:, S:2 * S],
                                 start=False, stop=True)

                # weighted sum: acc += alpha * histT
                tmp = tmpp.tile([P, 2 * S], FPR, tag="tmp")
                nc.vector.tensor_tensor(
                    out=tmp[:].rearrange("p (k s) -> p k s", k=2),
                    in0=hh[:].rearrange("p (k s) -> p k s", k=2),
                    in1=pal[:].unsqueeze(1).broadcast_to((P, 2, S)),
                    op=MUL,
                )
                nc.vector.tensor_tensor(
                    out=acc[:, 0:ACC_SPLIT], in0=acc[:, 0:ACC_SPLIT],
                    in1=tmp[:, 0:ACC_SPLIT], op=ADD)
                nc.gpsimd.tensor_tensor(
                    out=acc[:, ACC_SPLIT:], in0=acc[:, ACC_SPLIT:],
                    in1=tmp[:, ACC_SPLIT:], op=ADD)

            # ---- epilogue ---------------------------------------------------
            pfin = palp.tile([E, S], FP, tag="pal")
            nc.tensor.matmul(pfin[:], wf1_t[0][:], cur_t[0][:],
                             start=True, stop=False)
            nc.tensor.matmul(pfin[:], wf1_t[1][:], cur_t[1][:],
                             start=False, stop=False)
            nc.tensor.matmul(pfin[:], wf2_t[0][:], acc[:, 0:S],
                             start=False, stop=False)
            nc.tensor.matmul(pfin[:], wf2_t[1][:], acc[:, S:2 * S],
                             start=False, stop=True)
            osb = wp.tile([E, S], FP, tag="osb")
            nc.vector.tensor_tensor(out=osb[:], in0=pfin[:], in1=c0_t[:],
                                    op=ADD)
            nc.sync.dma_start(outT[:, :], osb[:])

    nc.compile()
    return nc


def _get_program():
    if "nc" not in _PROG_CACHE:
        _PROG_CACHE["nc"] = _build_program()
    return _PROG_CACHE["nc"]


def prepare_in_maps(wca_history_embeddings, wca_current_embeddings,
                    wc_w, wc_b, wh_w, wh_b, qt_w, qt_b, wf_w, wf_b):
    import ml_dtypes
    bf16 = ml_dtypes.bfloat16
    hist = np.asarray(wca_history_embeddings, dtype=np.float32)
    cur = np.asarray(wca_current_embeddings, dtype=np.float32)
    f32 = lambda x: np.ascontiguousarray(np.asarray(x), dtype=np.float32)
    bf = lambda x: np.ascontiguousarray(np.asarray(x, dtype=np.float32)).astype(bf16)

    whT = bf(np.asarray(wh_w).T)
    wcT = f32(np.asarray(wc_w).T)
    qtB = bf(np.repeat(np.asarray(qt_w).reshape(D, 1), P, axis=1))
    wf1T = f32(np.asarray(wf_w)[:, :D].T)
    wf2T = f32(np.asarray(wf_w)[:, D:].T)
    biasCW = f32((np.asarray(wc_b) + np.asarray(wh_b)).reshape(D, 1))
    identity = bf(np.eye(P))
    qtb = float(np.asarray(qt_b).reshape(-1)[0])
    wfb = np.asarray(wf_b, dtype=np.float64)

    in_maps = []
    for b in range(B):
        histT_b = np.ascontiguousarray(
            hist[b].transpose(0, 2, 1), dtype=np.float32
        ).astype(bf16).reshape(H, 2, P, S)
        curT_b = f32(cur[b].T)
        hs0 = hist[b].astype(np.float64).sum(axis=0)          # [S, D]
        c0 = qtb * (hs0 @ wf2T.astype(np.float64)) + wfb      # [S, E]
        c0T_b = f32(c0.T)
        in_maps.append({
            "histT": histT_b, "curT": curT_b, "whT": whT, "wcT": wcT,
            "qtB": qtB, "wf1T": wf1T, "wf2T": wf2T, "biasCW": biasCW,
            "ident": identity, "c0T": c0T_b,
        })
    return in_maps


def run_on_device(in_maps, trace=False, **kwargs):
    from concourse import bass_utils
    nc = _get_program()
    res = bass_utils.run_bass_kernel_spmd(
        nc, in_maps, core_ids=list(range(N_CORES)), trace=trace, **kwargs)
    return res


def kernel(wca_history_embeddings, wca_current_embeddings,
           wc_w, wc_b, wh_w, wh_b, qt_w, qt_b, wf_w, wf_b):
    in_maps = prepare_in_maps(
        wca_history_embeddings, wca_current_embeddings,
        wc_w, wc_b, wh_w, wh_b, qt_w, qt_b, wf_w, wf_b)
    res = run_on_device(in_maps)
    out = np.empty((B, S, E), dtype=np.float32)
    for b in range(B):
        out[b] = res.results[b]["outT"].T
    return out


# revision 7
# speedup vs baseline: 1.1289x; 1.0002x over previous
"""Trainium2 Bass kernel for nn_InformationAggregationLayer.

Math (per batch b, reference semantics):
  wc_out = cur @ wc_w.T + wc_b                      [S, D]
  wh_out = hist_h @ wh_w.T + wh_b                   [S, D]  per h
  comb   = sigmoid(wc_out + wh_out)                 [S, D]
  alpha  = comb @ qt_w.T + qt_b                     [S, 1]
  HS     = sum_h alpha_h * hist_h                   [S, D]
  out    = cur @ wf1.T + HS @ wf2.T + wf_b          [S, E]
         (wf1 = wf_w[:, :D], wf2 = wf_w[:, D:])

Device strategy (one NeuronCore per batch, 8 cores):
  - All tensors on chip in TRANSPOSED layout [feature, seq] so the
    feature contraction dim sits on SBUF partitions for matmuls.
  - hist fed pre-transposed from host: histT [H, D, S].
  - Per h: PE computes wh_outT into PSUM (fp32r), injects the
    precomputed (wc_out + wc_b + wh_b)T via an identity matmul into the
    same accumulation group, ACT applies sigmoid PSUM->SBUF, PE contracts
    with a broadcast-stationary qt (every output partition gets the same
    alpha row), then DVE/Pool compute HS += alpha * histT.
  - qt_b is folded into a host-precomputed epilogue constant:
      sum_h (a_h + qt_b) * hist_h @ wf2.T
        = [sum_h a_h*hist_h] @ wf2.T + qt_b * (sum_h hist_h) @ wf2.T,
    and the second term (+ wf_b) is c0T, uploaded per batch.
"""

import numpy as np

B, H, S, D, E = 8, 50, 512, 256, 128
P = 128
N_CORES = 8

_PROG_CACHE = {}

# Engine split for the weighted-sum accumulate: columns [0:ACC_SPLIT) of the
# [P, 2*S] acc tile are added on DVE, the rest on GpSimd (Pool).
ACC_SPLIT = 416


def _build_program():
    import concourse.bacc as bacc
    import concourse.tile as tile
    import concourse.mybir as mybir

    FP = mybir.dt.float32
    FPR = mybir.dt.float32r
    BF = mybir.dt.bfloat16
    SIG = mybir.ActivationFunctionType.Sigmoid
    ADD = mybir.AluOpType.add
    MUL = mybir.AluOpType.mult

    nc = bacc.Bacc("TRN2", target_bir_lowering=False, debug=False,
                   num_devices=N_CORES)

    # Per-core inputs. histT is viewed [H, 2, 128, S] so each partition of the
    # on-chip [128, 2*S] tile pulls its two d-slices. Matmul operands are
    # float32r (1 cycle/row on the PE vs 4 for plain fp32); walrus requires
    # every producer of an fp32r matmul operand to emit fp32r.
    histT = nc.dram_tensor("histT", [H, 2, P, S], BF, kind="ExternalInput")
    curT = nc.dram_tensor("curT", [D, S], FPR, kind="ExternalInput")
    whT = nc.dram_tensor("whT", [D, D], BF, kind="ExternalInput")
    wcT = nc.dram_tensor("wcT", [D, D], FPR, kind="ExternalInput")
    qtB = nc.dram_tensor("qtB", [D, P], BF, kind="ExternalInput")
    wf1T = nc.dram_tensor("wf1T", [D, E], FPR, kind="ExternalInput")
    wf2T = nc.dram_tensor("wf2T", [D, E], FPR, kind="ExternalInput")
    biasCW = nc.dram_tensor("biasCW", [D, 1], FP, kind="ExternalInput")
    ident = nc.dram_tensor("ident", [P, P], BF, kind="ExternalInput")
    c0T = nc.dram_tensor("c0T", [E, S], FP, kind="ExternalInput")
    outT = nc.dram_tensor("outT", [E, S], FP, kind="ExternalOutput")

    with tile.TileContext(nc) as tc:
        with (
            tc.tile_pool(name="w", bufs=1) as wp,
            tc.tile_pool(name="hh", bufs=4) as hhp,
            tc.tile_pool(name="comb", bufs=3) as combp,
            tc.tile_pool(name="tmp", bufs=3) as tmpp,
            tc.tile_pool(name="acc", bufs=1) as accp,
            tc.tile_pool(name="pwh", bufs=3, space="PSUM") as pwh,
            tc.tile_pool(name="pal", bufs=2, space="PSUM") as palp,
        ):
            # ---- weights / constants to SBUF -------------------------------
            def load(name, dram_ap, shape, dt=FPR):
                t = wp.tile(shape, dt, tag=name)
                nc.sync.dma_start(t[:], dram_ap)
                return t

            whT_t = [load(f"whT{k}", whT[k * P:(k + 1) * P, :], [P, D], dt=BF)
                     for k in range(2)]
            wcT_t = [load(f"wcT{k}", wcT[k * P:(k + 1) * P, :], [P, D])
                     for k in range(2)]
            qtB_t = [load(f"qtB{k}", qtB[k * P:(k + 1) * P, :], [P, P], dt=BF)
                     for k in range(2)]
            wf1_t = [load(f"wf1{k}", wf1T[k * P:(k + 1) * P, :], [P, E])
                     for k in range(2)]
            wf2_t = [load(f"wf2{k}", wf2T[k * P:(k + 1) * P, :], [P, E])
                     for k in range(2)]
            cur_t = [load(f"cur{k}", curT[k * P:(k + 1) * P, :], [P, S])
                     for k in range(2)]
            bias_t = [load(f"bias{k}", biasCW[k * P:(k + 1) * P, :], [P, 1],
                           dt=FP)
                      for k in range(2)]
            id_t = load("ident", ident[:, :], [P, P], dt=BF)
            c0_t = load("c0T", c0T[:, :], [E, S], dt=FP)

            acc = accp.tile([P, 2 * S], FPR, tag="acc")
            nc.gpsimd.memset(acc[:].bitcast(FP), 0.0)

            # ---- prologue: wcoT = (cur @ wc.T + wc_b + wh_b)T --------------
            # transposed layout [dout, s]; m-th dout-tile lives in free cols
            # [m*S, (m+1)*S).
            wcps = pwh.tile([P, 2 * S], FP, tag="ps")
            for m in range(2):
                for k in range(2):
                    nc.tensor.matmul(
                        wcps[:, m * S:(m + 1) * S],
                        wcT_t[k][:, m * P:(m + 1) * P],
                        cur_t[k][:],
                        start=(k == 0), stop=(k == 1),
                    )
            wco = wp.tile([P, 2 * S], BF, tag="wco")
            for m in range(2):
                sl = slice(m * S, (m + 1) * S)
                nc.vector.tensor_scalar(
                    out=wco[:, sl], in0=wcps[:, sl],
                    scalar1=bias_t[m][:], scalar2=None, op0=ADD,
                )

            # ---- main loop over history ------------------------------------
            for h in range(H):
                hh = hhp.tile([P, 2 * S], BF, tag="hh")
                nc.sync.dma_start(hh[:, 0:S], histT[h, 0])
                nc.sync.dma_start(hh[:, S:2 * S], histT[h, 1])
                ps = pwh.tile([P, 2 * S], FP, tag="ps")
                for m in range(2):
                    sl = slice(m * S, (m + 1) * S)
                    nc.tensor.matmul(
                        ps[:, sl],
                        whT_t[0][:, m * P:(m + 1) * P],
                        hh[:, 0:S],
                        start=True, stop=False,
                    )
                    nc.tensor.matmul(
                        ps[:, sl],
                        whT_t[1][:, m * P:(m + 1) * P],
                        hh[:, S:2 * S],
                        start=False, stop=False,
                    )
                    # inject wc_out + biases (identity stationary)
                    nc.tensor.matmul(
                        ps[:, sl],
                        id_t[:],
                        wco[:, sl],
                        start=False, stop=True,
                    )
                comb = combp.tile([P, 2 * S], BF, tag="comb")
                nc.scalar.activation(comb[:], ps[:], SIG)

                # alpha (without qt_b), broadcast to all 128 partitions:
                # stationary qtB[k][i, m] = qt[k*128+i] for every m.
                pal = palp.tile([P, S], FP, tag="pal")
                nc.tensor.matmul(pal[:], qtB_t[0][:], comb[:, 0:S],
                                 start=True, stop=False)
                nc.tensor.matmul(pal[:], qtB_t[1][:], comb[:, S:2 * S],
                                 start=False, stop=True)

                # weighted sum: acc += alpha * histT. Cast alpha to bf16 so
                # the multiply runs in the DVE 2x packed mode (both operands
                # 16-bit); accumulate stays fp32, split across DVE and Pool.
                palb = tmpp.tile([P, S], BF, tag="palb")
                nc.vector.tensor_copy(palb[:], pal[:])
                tmp = tmpp.tile([P, 2 * S], BF, tag="tmp")
                nc.vector.tensor_tensor(
                    out=tmp[:].rearrange("p (k s) -> p k s", k=2),
                    in0=hh[:].rearrange("p (k s) -> p k s", k=2),
                    in1=palb[:].unsqueeze(1).broadcast_to((P, 2, S)),
                    op=MUL,
                )
                nc.vector.tensor_tensor(
                    out=acc[:, 0:ACC_SPLIT], in0=acc[:, 0:ACC_SPLIT],
                    in1=tmp[:, 0:ACC_SPLIT], op=ADD)
                nc.gpsimd.tensor_tensor(
                    out=acc[:, ACC_SPLIT:], in0=acc[:, ACC_SPLIT:],
                    in1=tmp[:, ACC_SPLIT:], op=ADD)

            # ---- epilogue ---------------------------------------------------
            pfin = palp.tile([E, S], FP, tag="pal")
            nc.tensor.matmul(pfin[:], wf1_t[0][:], cur_t[0][:],
                             start=True, stop=False)
            nc.tensor.matmul(pfin[:], wf1_t[1][:], cur_t[1][:],
                             start=False, stop=False)
            nc.tensor.matmul(pfin[:], wf2_t[0][:], acc[:, 0:S],
                             start=False, stop=False)
            nc.tensor.matmul(pfin[:], wf2_t[1][:], acc[:, S:2 * S],
                             start=False, stop=True)
            osb = wp.tile([E, S], FP, tag="osb")
            nc.vector.tensor_tensor(out=osb[:], in0=pfin[:], in1=c0_t[:],
                                    op=ADD)
            nc.sync.dma_start(outT[:, :], osb[:])

    nc.compile()
    return nc


def _get_program():
    if "nc" not in _PROG_CACHE:
        _PROG_CACHE["nc"] = _build_program()
    return _PROG_CACHE["nc"]


def prepare_in_maps(wca_history_embeddings, wca_current_embeddings,
                    wc_w, wc_b, wh_w, wh_b, qt_w, qt_b, wf_w, wf_b):
    import ml_dtypes
    bf16 = ml_dtypes.bfloat16
    hist = np.asarray(wca_history_embeddings, dtype=np.float32)
    cur = np.asarray(wca_current_embeddings, dtype=np.float32)
    f32 = lambda x: np.ascontiguousarray(np.asarray(x), dtype=np.float32)
    bf = lambda x: np.ascontiguousarray(np.asarray(x, dtype=np.float32)).astype(bf16)

    whT = bf(np.asarray(wh_w).T)
    wcT = f32(np.asarray(wc_w).T)
    qtB = bf(np.repeat(np.asarray(qt_w).reshape(D, 1), P, axis=1))
    wf1T = f32(np.asarray(wf_w)[:, :D].T)
    wf2T = f32(np.asarray(wf_w)[:, D:].T)
    biasCW = f32((np.asarray(wc_b) + np.asarray(wh_b)).reshape(D, 1))
    identity = bf(np.eye(P))
    qtb = float(np.asarray(qt_b).reshape(-1)[0])
    wfb = np.asarray(wf_b, dtype=np.float64)

    in_maps = []
    for b in range(B):
        histT_b = np.ascontiguousarray(
            hist[b].transpose(0, 2, 1), dtype=np.float32
        ).astype(bf16).reshape(H, 2, P, S)
        curT_b = f32(cur[b].T)
        hs0 = hist[b].astype(np.float64).sum(axis=0)          # [S, D]
        c0 = qtb * (hs0 @ wf2T.astype(np.float64)) + wfb      # [S, E]
        c0T_b = f32(c0.T)
        in_maps.append({
            "histT": histT_b, "curT": curT_b, "whT": whT, "wcT": wcT,
            "qtB": qtB, "wf1T": wf1T, "wf2T": wf2T, "biasCW": biasCW,
            "ident": identity, "c0T": c0T_b,
        })
    return in_maps


def run_on_device(in_maps, trace=False, **kwargs):
    from concourse import bass_utils
    nc = _get_program()
    res = bass_utils.run_bass_kernel_spmd(
        nc, in_maps, core_ids=list(range(N_CORES)), trace=trace, **kwargs)
    return res


def kernel(wca_history_embeddings, wca_current_embeddings,
           wc_w, wc_b, wh_w, wh_b, qt_w, qt_b, wf_w, wf_b):
    in_maps = prepare_in_maps(
        wca_history_embeddings, wca_current_embeddings,
        wc_w, wc_b, wh_w, wh_b, qt_w, qt_b, wf_w, wf_b)
    res = run_on_device(in_maps)
    out = np.empty((B, S, E), dtype=np.float32)
    for b in range(B):
        out[b] = res.results[b]["outT"].T
    return out
